# revision 1
# baseline (speedup 1.0000x reference)
"""Matcher kernel v2 — stride-0-source DMA design.

Per (t, g) tile on each core:
  ch0 (p1 rows broadcast along j): expand p1 [82, 256B] -> e0 [82, r*256B]
      on DVE (log-doubling), then SWDGE DMA with stride-0 middle dim writes
      [m1, m2, K]: descriptors r*256B, the empirically fast shape
      (~145 GB/s/core vs ~42 for contiguous-source ops).
  ch1 (p2 block repeated on every row): load the block's 8 chunks onto
      4*8 partitions (4 replicas), then 4 stride-0-leading DMAs write
      consecutive row blocks; engine-free.

All big DMAs ride the gpsimd (SWDGE) queue; sync/scalar HWDGE queues
carry only the small input loads (HWDGE measured ~27 GB/s — too slow
for bulk).
"""

import sys

if '/opt/trn_rl_repo' not in sys.path:
    sys.path.insert(0, '/opt/trn_rl_repo')

import numpy as np

G, N1, N2, K = 32, 2048, 2048, 64
N_CORES = 8
GPC = G // N_CORES
NT = 3
R0 = 16          # ch0 pre-expansion reps (descriptor = R0*256B = 4 KB)
NCH = 8          # ch1 block chunks across partitions
NREP = 4         # ch1 replica sets / row blocks

_cache = {}


def _build(m1, m2):
    from concourse import bacc
    import concourse.tile as tile
    import concourse.mybir as mybir

    F32 = mybir.dt.float32
    m2k = m2 * K
    chw = m2k // NCH
    chr_ = m2k - chw * (NCH - 1)   # last chunk may differ if NCH ∤ m2k
    assert m2k % NCH == 0, (m2, m2k)
    nrep0 = m2 // R0
    rem0 = m2 - nrep0 * R0
    row_blocks = []
    base, step = 0, (m1 + NREP - 1) // NREP
    for k in range(NREP):
        row_blocks.append((base, min(base + step, m1)))
        base = min(base + step, m1)

    nc = bacc.Bacc("TRN2", target_bir_lowering=False, debug=False)
    p1 = nc.dram_tensor("p1", [NT, GPC, m1, K], F32, kind="ExternalInput")
    p2f = nc.dram_tensor("p2f", [NT, GPC, 1, m2k], F32, kind="ExternalInput")
    out = nc.dram_tensor("out", [NT, GPC, 2, m1, m2k], F32,
                         kind="ExternalOutput")

    with tile.TileContext(nc) as tc:
        with tc.tile_pool(name="e0", bufs=3) as pe0, \
             tc.tile_pool(name="e1", bufs=3) as pe1, \
             tc.tile_pool(name="in1", bufs=3) as pin1:
            for t in range(NT):
                for g in range(GPC):
                    # ---- ch0 ----
                    t1 = pin1.tile([m1, K], F32)
                    nc.sync.dma_start(t1[:], p1[t, g])
                    e0 = pe0.tile([m1, R0 * K], F32)
                    nc.vector.tensor_copy(e0[:, :K], t1[:])
                    reps = 1
                    while reps < R0:
                        n = min(reps, R0 - reps)
                        nc.vector.tensor_copy(
                            e0[:, reps * K:(reps + n) * K], e0[:, :n * K])
                        reps += n
                    dst0 = out[t, g, 0][:, :nrep0 * R0 * K].rearrange(
                        "p (a b) -> p a b", a=nrep0)
                    src0 = e0[:].unsqueeze(1).broadcast_to(
                        [m1, nrep0, R0 * K])
                    nc.gpsimd.dma_start(dst0, src0)
                    if rem0:
                        nc.gpsimd.dma_start(
                            out[t, g, 0][:, nrep0 * R0 * K:],
                            e0[:, :rem0 * K])

                    # ---- ch1 ----
                    e1 = pe1.tile([NREP * NCH, chw], F32)
                    blk = p2f[t, g].squeeze(0).rearrange(
                        "(c w) -> c w", c=NCH)
                    # one plain load per replica group: a dst AP with two
                    # partition-step dims mis-lowers (writes stray SBUF)
                    for rr in range(NREP):
                        nc.sync.dma_start(e1[rr * NCH:(rr + 1) * NCH], blk)
                    ch1_qs = [nc.gpsimd, nc.sync, nc.gpsimd, nc.scalar]
                    for k, (lo, hi) in enumerate(row_blocks):
                        # SBUF src must be partition-leading; stride-0
                        # (row-repeat) dim second, dst chunk-major.
                        dst1 = out[t, g, 1][lo:hi].rearrange(
                            "p (c w) -> c p w", c=NCH)
                        src1 = e1[k * NCH:(k + 1) * NCH].unsqueeze(
                            1).broadcast_to([NCH, hi - lo, chw])
                        ch1_qs[k % len(ch1_qs)].dma_start(dst1, src1)
    nc.compile()
    return nc


def _pad_groups_np(x, ids, m):
    counts = np.bincount(ids, minlength=G)
    starts = np.cumsum(counts) - counts
    pos = np.arange(ids.shape[0]) - starts[ids]
    outp = np.zeros((G, m, x.shape[1]), x.dtype)
    outp[ids, pos] = x
    return outp


def _make_in_maps(inputs):
    ids1 = np.asarray(inputs['ids1']).astype(np.int64)
    ids2 = np.asarray(inputs['ids2']).astype(np.int64)
    m1 = int(inputs['maxcount1'])
    m2 = int(inputs['maxcount2'])
    xs1 = [np.asarray(inputs[n], dtype=np.float32)
           for n in ('x_f_1', 'x_e_1', 'x_v_1')]
    xs2 = [np.asarray(inputs[n], dtype=np.float32)
           for n in ('x_f_2', 'x_e_2', 'x_v_2')]
    pad1 = np.stack([_pad_groups_np(x, ids1, m1) for x in xs1])
    pad2f = np.stack([_pad_groups_np(x, ids2, m2) for x in xs2]).reshape(
        NT, G, 1, m2 * K)
    in_maps = [
        {"p1": np.ascontiguousarray(pad1[:, c * GPC:(c + 1) * GPC]),
         "p2f": np.ascontiguousarray(pad2f[:, c * GPC:(c + 1) * GPC])}
        for c in range(N_CORES)
    ]
    return in_maps, m1, m2


def kernel(**inputs):
    from concourse.bass_utils import run_bass_kernel_spmd

    in_maps, m1, m2 = _make_in_maps(inputs)
    key = (m1, m2)
    if key not in _cache:
        _cache[key] = _build(m1, m2)
    nc = _cache[key]

    res = run_bass_kernel_spmd(nc, in_maps, core_ids=list(range(N_CORES)))

    full = np.empty((NT, G, 2, m1, m2, K), np.float32)
    for c in range(N_CORES):
        full[:, c * GPC:(c + 1) * GPC] = res.results[c]["out"].reshape(
            NT, GPC, 2, m1, m2, K)
    return full[0], full[1], full[2]

